# revision 11
# baseline (speedup 1.0000x reference)
"""Trainium2 Bass kernel for the masked-attention module.

Math (per batch row b):
    att_h = h @ W_h2att.T + b_h2att                       # [A]
    dot_l = sum_a tanh(f2[l,a] + att_h[a]) * w_alpha[a]   # [L]  (b_alpha cancels)
    E     = exp(dot)                                      # softmax numerator;
                                                          # denominator cancels with
                                                          # the masked renorm
    w     = E*mask / sum(E*mask)
    out   = sum_l w[l] * f1[l,:]                          # [D]

Sharding: data-parallel over B across 8 NeuronCores (16 rows each);
weights replicated.  Everything runs in fp32.
"""

import numpy as np

import concourse.bacc as bacc
import concourse.bass as bass
import concourse.mybir as mybir
import concourse.tile as tile
from concourse.bass import ts
from concourse.bass_utils import run_bass_kernel_spmd
from concourse.masks import make_identity

# Problem geometry (hardcoded per spec).
B, L, RNN, ATT = 128, 1024, 1024, 512
N_CORES = 8
BS = B // N_CORES          # 16 batch rows per core
P = 128                    # partitions
F32 = mybir.dt.float32
AF = mybir.ActivationFunctionType
ALU = mybir.AluOpType


def build_nc(BS=B // N_CORES, L=L, RNN=RNN, ATT=ATT):
    LC = L // P            # l-chunks
    RC = RNN // P          # r-chunks
    AC = ATT // P          # a-chunks
    nc = bacc.Bacc("TRN2", target_bir_lowering=False, debug=False)

    h_d = nc.dram_tensor("h", [BS, RNN], F32, kind="ExternalInput").ap()
    f1_d = nc.dram_tensor("att_feats1", [BS, L, RNN], F32, kind="ExternalInput").ap()
    f2_d = nc.dram_tensor("att_feats2", [BS, L, ATT], F32, kind="ExternalInput").ap()
    mask_d = nc.dram_tensor("att_masks", [BS, L], F32, kind="ExternalInput").ap()
    w_d = nc.dram_tensor("W_h2att", [ATT, RNN], F32, kind="ExternalInput").ap()
    bh_d = nc.dram_tensor("b_h2att", [ATT], F32, kind="ExternalInput").ap()
    wa_d = nc.dram_tensor("w_alpha", [ATT], F32, kind="ExternalInput").ap()
    out_d = nc.dram_tensor("out", [BS, RNN], F32, kind="ExternalOutput").ap()

    with tile.TileContext(nc) as tc:
        with (
            tc.tile_pool(name="singles", bufs=1) as singles,
            tc.tile_pool(name="wn", bufs=2) as wn_pool,
            tc.tile_pool(name="f2", bufs=4) as f2_pool,
            tc.tile_pool(name="f1", bufs=6) as f1_pool,
            tc.tile_pool(name="work", bufs=3) as work_pool,
            tc.tile_pool(name="psum_misc", bufs=2, space="PSUM") as psum_misc,
            tc.tile_pool(name="psum_ahb", bufs=2, space="PSUM") as psum_ahb_pool,
            tc.tile_pool(name="psum_out", bufs=2, space="PSUM") as psum_out_pool,
        ):
            # ---------- constants ----------
            identity = singles.tile([P, P], F32)
            make_identity(nc, identity[:])
            ones_row = singles.tile([1, P], F32)   # [1,128] of 1.0 (K=1 bcast mm)
            nc.vector.memset(ones_row[:], 1.0)
            ones_col = singles.tile([P, 1], F32)   # [128,1] of 1.0 (partition sum)
            nc.vector.memset(ones_col[:], 1.0)

            wa_bcast = singles.tile([P, ATT], F32)
            nc.sync.dma_start(wa_bcast[:], wa_d[None, :].to_broadcast((P, ATT)))
            bh_sb = singles.tile([1, ATT], F32)
            nc.sync.dma_start(bh_sb[:], bh_d[None, :])

            # ---------- prologue: att_h = h @ W.T + bh ----------
            # W^T in SBUF: WT[:, rc*ATT + a] = W[a, rc*128 + p]
            wt_all = singles.tile([P, RC * ATT], F32)
            for ac in range(AC):
                wn = wn_pool.tile([P, RNN], F32)
                nc.sync.dma_start(wn[:], w_d[ts(ac, P)])
                for rc in range(RC):
                    pt = psum_misc.tile([P, P], F32, tag="misc")
                    nc.tensor.transpose(pt[:], wn[:, ts(rc, P)], identity[:])
                    nc.scalar.copy(
                        wt_all[:, rc * ATT + ac * P : rc * ATT + (ac + 1) * P], pt[:]
                    )

            # h^T in SBUF: hT[:, rc*BS + b] = h[b, rc*128 + p]
            hn = singles.tile([BS, RNN], F32)
            nc.sync.dma_start(hn[:], h_d[:])
            ht_all = singles.tile([P, RC * BS], F32)
            for rc in range(RC):
                pt = psum_misc.tile([P, BS], F32, tag="misc")
                nc.tensor.transpose(pt[:], hn[:, ts(rc, P)], identity[:BS, :BS])
                nc.scalar.copy(ht_all[:, ts(rc, BS)], pt[:])

            # att_h rows, all staged on partition 0: ah_rows[0, b*ATT + a]
            ah_rows = singles.tile([1, BS * ATT], F32)
            for b in range(BS):
                ah_ps = psum_misc.tile([1, ATT], F32, tag="misc")
                for rc in range(RC):
                    nc.tensor.matmul(
                        ah_ps[:],
                        ht_all[:, rc * BS + b : rc * BS + b + 1],
                        wt_all[:, ts(rc, ATT)],
                        start=(rc == 0),
                        stop=False,
                    )
                # + b_h2att via K=1 matmul of ones
                nc.tensor.matmul(
                    ah_ps[:], ones_row[:, :1], bh_sb[:], start=False, stop=True
                )
                nc.scalar.copy(ah_rows[:, ts(b, ATT)], ah_ps[:])

            # ---------- mask transpose: maskT[p, b*LC+c] = mask[b, c*128+p] ----------
            mask_nat = singles.tile([BS, L], F32)
            nc.sync.dma_start(mask_nat[:], mask_d[:])
            maskT = singles.tile([P, BS * LC], F32)
            maskT_3d = maskT[:].rearrange("p (b c) -> p b c", c=LC)
            for c in range(LC):
                pt = psum_misc.tile([P, BS], F32, tag="misc")
                nc.tensor.transpose(pt[:], mask_nat[:, ts(c, P)], identity[:BS, :BS])
                nc.vector.tensor_copy(maskT_3d[:, :, c], pt[:])

            # ---------- phase 1: dots ----------
            dot_all = singles.tile([P, BS * LC], F32)
            for b in range(BS):
                # broadcast att_h[b,:] to all 128 partitions via K=1 matmul
                ahb = psum_ahb_pool.tile([P, ATT], F32, tag="ahb")
                nc.tensor.matmul(
                    ahb[:], ones_row[:], ah_rows[:, ts(b, ATT)], start=True, stop=True
                )
                for c in range(LC):
                    f2t = f2_pool.tile([P, ATT], F32, tag="f2")
                    nc.sync.dma_start(f2t[:], f2_d[b, ts(c, P)])
                    t_add = work_pool.tile([P, ATT], F32, tag="tadd")
                    nc.vector.tensor_add(t_add[:], f2t[:], ahb[:])
                    t_tanh = work_pool.tile([P, ATT], F32, tag="ttanh")
                    nc.scalar.activation(t_tanh[:], t_add[:], AF.Tanh)
                    scratch = work_pool.tile([P, ATT], F32, tag="scratch")
                    # fused multiply(+w_alpha) + free-dim reduce on DVE;
                    # the native TENSOR_TENSOR_REDUCE opcode crashes TRN2,
                    # scalar_tensor_tensor's accum_out path works.
                    nc.vector.scalar_tensor_tensor(
                        out=scratch[:],
                        in0=t_tanh[:],
                        scalar=1.0,
                        in1=wa_bcast[:],
                        op0=ALU.mult,
                        op1=ALU.mult,
                        accum_out=dot_all[:, b * LC + c : b * LC + c + 1],
                    )

            # ---------- phase 2: masked softmax (denominator cancels) ----------
            e_all = singles.tile([P, BS * LC], F32)
            nc.scalar.activation(e_all[:], dot_all[:], AF.Exp)
            m_all = singles.tile([P, BS * LC], F32)
            nc.vector.tensor_mul(m_all[:], e_all[:], maskT[:])
            # per-(p, b) partial sums over the 8 l-chunks
            s_pb = singles.tile([P, BS], F32)
            nc.vector.tensor_reduce(
                s_pb[:],
                m_all[:].rearrange("p (b c) -> p b c", c=LC),
                axis=mybir.AxisListType.X,
                op=ALU.add,
            )
            # partition sum -> [1, BS]
            ssum_ps = psum_misc.tile([1, BS], F32, tag="misc")
            nc.tensor.matmul(ssum_ps[:], ones_col[:], s_pb[:], start=True, stop=True)
            ssum_sb = singles.tile([1, BS], F32)
            nc.vector.tensor_copy(ssum_sb[:], ssum_ps[:])
            rsum_sb = singles.tile([1, BS], F32)
            nc.vector.reciprocal(rsum_sb[:], ssum_sb[:])
            # broadcast 1/sum to all partitions
            rb_ps = psum_misc.tile([P, BS], F32, tag="misc")
            nc.tensor.matmul(rb_ps[:], ones_row[:], rsum_sb[:], start=True, stop=True)
            w_all = singles.tile([P, BS * LC], F32)
            nc.vector.tensor_mul(
                w_all[:].rearrange("p (b c) -> p b c", c=LC),
                m_all[:].rearrange("p (b c) -> p b c", c=LC),
                rb_ps[:][:, :, None].to_broadcast((P, BS, LC)),
            )

            # ---------- phase 3: out[b,:] = sum_l w[l] * f1[b,l,:] ----------
            # staged on partition 0 as [1, BS*RNN]; engines cannot move data
            # across partitions, so keep everything on partition 0 and let the
            # final DMA scatter it to DRAM.
            att_out = singles.tile([1, BS * RNN], F32)
            for b in range(BS):
                o_ps = psum_out_pool.tile([1, RNN], F32, tag="out")
                d_chunk = min(512, RNN)
                for c in range(LC):
                    f1t = f1_pool.tile([P, RNN], F32, tag="f1")
                    nc.sync.dma_start(f1t[:], f1_d[b, ts(c, P)])
                    w_col = w_all[:, b * LC + c : b * LC + c + 1]
                    for dc in range(RNN // d_chunk):
                        nc.tensor.matmul(
                            o_ps[:, ts(dc, d_chunk)],
                            w_col,
                            f1t[:, ts(dc, d_chunk)],
                            start=(c == 0),
                            stop=(c == LC - 1),
                        )
                nc.scalar.copy(att_out[:, ts(b, RNN)], o_ps[:])

            nc.sync.dma_start(out_d.rearrange("b d -> (b d)")[None, :], att_out[:])

    nc.compile()
    return nc


_NC_CACHE = None


def _get_nc():
    global _NC_CACHE
    if _NC_CACHE is None:
        _NC_CACHE = build_nc()
    return _NC_CACHE


def _make_in_maps(inputs):
    f32 = lambda x: np.ascontiguousarray(np.asarray(x, dtype=np.float32))
    h = f32(inputs["h"])
    f1 = f32(inputs["att_feats1"])
    f2 = f32(inputs["att_feats2"])
    mask = f32(inputs["att_masks"])
    w = f32(inputs["W_h2att"])
    bh = f32(inputs["b_h2att"])
    wa = f32(inputs["w_alpha"])
    in_maps = []
    for i in range(N_CORES):
        sl = slice(i * BS, (i + 1) * BS)
        in_maps.append(
            {
                "h": h[sl],
                "att_feats1": f1[sl],
                "att_feats2": f2[sl],
                "att_masks": mask[sl],
                "W_h2att": w,
                "b_h2att": bh,
                "w_alpha": wa,
            }
        )
    return in_maps


def _ensure_ntff_hook():
    """The agent image's antenv lacks axon_hooks; shim it so trace=True can
    capture NTFF profiles through libaxon_pjrt's ctypes interface."""
    import sys
    import types

    try:
        import antenv.axon_hooks  # noqa: F401
        return
    except ImportError:
        pass
    try:
        from trn_agent_boot.trn_boot import _ntff_profile_via_ctypes

        hook = _ntff_profile_via_ctypes("/opt/axon/libaxon_pjrt.so")
    except Exception:
        hook = None
    mod = types.ModuleType("antenv.axon_hooks")
    mod._hook = hook
    mod.get_axon_ntff_profile_hook = lambda: mod._hook
    mod.set_axon_ntff_profile_hook = lambda h: setattr(mod, "_hook", h)
    sys.modules["antenv.axon_hooks"] = mod


def run(inputs, trace=False):
    """Returns (full_output [B, RNN] float32, exec_time_ns or None)."""
    if trace:
        _ensure_ntff_hook()
    nc = _get_nc()
    res = run_bass_kernel_spmd(
        nc, _make_in_maps(inputs), core_ids=list(range(N_CORES)), trace=trace
    )
    out = np.concatenate([r["out"] for r in res.results], axis=0)
    return out.astype(np.float32), res.exec_time_ns


def kernel(**inputs):
    out, _ = run(inputs, trace=False)
    return out


# revision 14
# speedup vs baseline: 2.3492x; 2.3492x over previous
"""Trainium2 Bass kernel for the masked-attention module.

Math (per batch row b):
    att_h = h @ W_h2att.T + b_h2att                       # [A]
    dot_l = sum_a tanh(f2[l,a] + att_h[a]) * w_alpha[a]   # [L]  (b_alpha cancels)
    m     = exp(dot) * mask      # softmax denominator cancels with masked renorm
    out   = (sum_l m[l] * f1[l,:]) / sum_l m[l]           # [D]

Sharding: data-parallel over B across 8 NeuronCores (16 rows each);
weights replicated.  Inputs are downcast to bf16 on the host (fp32 matmuls
run at 1/4 rate on TRN2 and fp32 doubles the DMA bytes); accumulations are
fp32.  Verified rel err vs the fp32 reference ~2.4e-3.
"""

import numpy as np

import concourse.bacc as bacc
import concourse.bass as bass
import concourse.mybir as mybir
import concourse.tile as tile
from concourse.bass import ts
from concourse.bass_utils import run_bass_kernel_spmd
from concourse.masks import make_identity

# Problem geometry (hardcoded per spec).
B, L, RNN, ATT = 128, 1024, 1024, 512
N_CORES = 8
BS = B // N_CORES          # 16 batch rows per core
P = 128                    # partitions
F32 = mybir.dt.float32
BF16 = mybir.dt.bfloat16
AF = mybir.ActivationFunctionType
ALU = mybir.AluOpType


def build_nc(BS=BS, L=L, RNN=RNN, ATT=ATT):
    LC = L // P            # l-chunks
    RC = RNN // P          # r-chunks
    AC = ATT // P          # a-chunks
    nc = bacc.Bacc("TRN2", target_bir_lowering=False, debug=False)

    h_d = nc.dram_tensor("h", [BS, RNN], BF16, kind="ExternalInput").ap()
    f1_d = nc.dram_tensor("att_feats1", [BS, L, RNN], BF16, kind="ExternalInput").ap()
    f2_d = nc.dram_tensor("att_feats2", [BS, L, ATT], BF16, kind="ExternalInput").ap()
    mask_d = nc.dram_tensor("att_masks", [BS, L], F32, kind="ExternalInput").ap()
    w_d = nc.dram_tensor("W_h2att", [ATT, RNN], BF16, kind="ExternalInput").ap()
    bh_d = nc.dram_tensor("b_h2att", [ATT], BF16, kind="ExternalInput").ap()
    wa_d = nc.dram_tensor("w_alpha", [ATT], BF16, kind="ExternalInput").ap()
    out_d = nc.dram_tensor("out", [BS, RNN], F32, kind="ExternalOutput").ap()

    with tile.TileContext(nc) as tc:
        with (
            tc.tile_pool(name="singles", bufs=1) as singles,
            tc.tile_pool(name="wn", bufs=2) as wn_pool,
            tc.tile_pool(name="f2", bufs=3) as f2_pool,
            tc.tile_pool(name="f1", bufs=3) as f1_pool,
            tc.tile_pool(name="work", bufs=2) as work_pool,
            tc.tile_pool(name="small", bufs=3) as small_pool,
            tc.tile_pool(name="psum_misc", bufs=2, space="PSUM") as psum_misc,
            tc.tile_pool(name="psum_ahb", bufs=2, space="PSUM") as psum_ahb_pool,
            tc.tile_pool(name="psum_out", bufs=2, space="PSUM") as psum_out_pool,
        ):
            # ---------- constants ----------
            ident_bf = singles.tile([P, P], BF16)
            make_identity(nc, ident_bf[:])
            ident_f32 = singles.tile([P, P], F32)
            make_identity(nc, ident_f32[:])
            ones_row = singles.tile([1, P], BF16)   # K=1 bcast matmuls
            nc.vector.memset(ones_row[:], 1.0)
            ones_col = singles.tile([P, 1], F32)    # partition sums
            nc.vector.memset(ones_col[:], 1.0)

            wa_bcast = singles.tile([P, ATT], BF16)
            nc.sync.dma_start(wa_bcast[:], wa_d[None, :].to_broadcast((P, ATT)))
            bh_sb = singles.tile([1, ATT], BF16)
            nc.sync.dma_start(bh_sb[:], bh_d[None, :])

            # ---------- prologue: att_h = h @ W.T + bh ----------
            # W^T in SBUF (bf16): WT[:, rc*ATT + a] = W[a, rc*128 + p]
            wt_all = singles.tile([P, RC * ATT], BF16)
            for ac in range(AC):
                wn = wn_pool.tile([P, RNN], BF16)
                nc.sync.dma_start(wn[:], w_d[ts(ac, P)])
                for rc in range(RC):
                    pt = psum_misc.tile([P, P], BF16, tag="misc")
                    nc.tensor.transpose(pt[:], wn[:, ts(rc, P)], ident_bf[:])
                    nc.scalar.copy(
                        wt_all[:, rc * ATT + ac * P : rc * ATT + (ac + 1) * P], pt[:]
                    )

            # h^T in SBUF (bf16): hT[:, rc*BS + b] = h[b, rc*128 + p]
            hn = singles.tile([BS, RNN], BF16)
            nc.sync.dma_start(hn[:], h_d[:])
            ht_all = singles.tile([P, RC * BS], BF16)
            for rc in range(RC):
                pt = psum_misc.tile([P, BS], BF16, tag="misc")
                nc.tensor.transpose(pt[:], hn[:, ts(rc, P)], ident_bf[:BS, :BS])
                nc.scalar.copy(ht_all[:, ts(rc, BS)], pt[:])

            # att_h rows, staged bf16 on partition 0: ah_rows[0, b*ATT + a]
            ah_rows = singles.tile([1, BS * ATT], BF16)
            for b in range(BS):
                ah_ps = psum_misc.tile([1, ATT], F32, tag="misc")
                for rc in range(RC):
                    nc.tensor.matmul(
                        ah_ps[:],
                        ht_all[:, rc * BS + b : rc * BS + b + 1],
                        wt_all[:, ts(rc, ATT)],
                        start=(rc == 0),
                        stop=False,
                    )
                nc.tensor.matmul(
                    ah_ps[:], ones_row[:, :1], bh_sb[:], start=False, stop=True
                )
                nc.scalar.copy(ah_rows[:, ts(b, ATT)], ah_ps[:])

            # ---------- mask transpose: maskT[p, b*LC+c] = mask[b, c*128+p] ----------
            mask_nat = singles.tile([BS, L], F32)
            nc.sync.dma_start(mask_nat[:], mask_d[:])
            maskT = singles.tile([P, BS * LC], F32)
            maskT_3d = maskT[:].rearrange("p (b c) -> p b c", c=LC)
            for c in range(LC):
                pt = psum_misc.tile([P, BS], F32, tag="misc")
                nc.tensor.transpose(pt[:], mask_nat[:, ts(c, P)], ident_f32[:BS, :BS])
                nc.vector.tensor_copy(maskT_3d[:, :, c], pt[:])

            dot_all = singles.tile([P, BS * LC], F32)
            att_out = singles.tile([1, BS * RNN], F32)

            # ---------- per-batch pipeline ----------
            for b in range(BS):
                # broadcast att_h[b,:] to all partitions (K=1 matmul), cast bf16
                ahb_ps = psum_ahb_pool.tile([P, ATT], F32, tag="ahb")
                nc.tensor.matmul(
                    ahb_ps[:], ones_row[:], ah_rows[:, ts(b, ATT)], start=True, stop=True
                )
                ahb = work_pool.tile([P, ATT], BF16, tag="ahb_sb")
                nc.scalar.copy(ahb[:], ahb_ps[:])

                # f2[b] in one 1 MiB DMA: [128, LC, ATT], l = c*128 + p
                f2t = f2_pool.tile([P, LC, ATT], BF16, tag="f2")
                nc.sync.dma_start(
                    f2t[:], f2_d[b].rearrange("(c p) a -> p c a", p=P)
                )
                # add att_h (whole-b op), tanh (whole-b op)
                t_add = work_pool.tile([P, LC, ATT], BF16, tag="tadd")
                nc.vector.tensor_add(
                    t_add[:],
                    f2t[:],
                    ahb[:][:, None, :].to_broadcast((P, LC, ATT)),
                )
                t_tanh = work_pool.tile([P, LC * ATT], BF16, tag="ttanh")
                nc.scalar.activation(
                    t_tanh[:], t_add[:].rearrange("p c a -> p (c a)"), AF.Tanh
                )
                # per-chunk fused multiply(+w_alpha) + free-dim reduce
                for c in range(LC):
                    scratch = work_pool.tile([P, ATT], BF16, tag="scratch")
                    nc.vector.scalar_tensor_tensor(
                        out=scratch[:],
                        in0=t_tanh[:, ts(c, ATT)],
                        scalar=1.0,
                        in1=wa_bcast[:],
                        op0=ALU.mult,
                        op1=ALU.mult,
                        accum_out=dot_all[:, b * LC + c : b * LC + c + 1],
                    )

                # masked exp (denominator cancels; normalization folded into
                # the output copy's scale)
                e_b = small_pool.tile([P, LC], F32, tag="eb")
                nc.scalar.activation(
                    e_b[:], dot_all[:, ts(b, LC)], AF.Exp
                )
                m_b = small_pool.tile([P, LC], F32, tag="mb")
                nc.vector.tensor_mul(m_b[:], e_b[:], maskT[:, ts(b, LC)])
                mw_b = small_pool.tile([P, LC], BF16, tag="mwb")
                nc.vector.tensor_copy(mw_b[:], m_b[:])
                # sum over l: free reduce then partition sum via ones-matmul
                s_b = small_pool.tile([P, 1], F32, tag="sb")
                nc.vector.tensor_reduce(
                    s_b[:], m_b[:], axis=mybir.AxisListType.X, op=ALU.add
                )
                ssum_ps = psum_misc.tile([1, 1], F32, tag="misc")
                nc.tensor.matmul(ssum_ps[:], ones_col[:], s_b[:], start=True, stop=True)
                rsum = small_pool.tile([1, 1], F32, tag="rsum")
                nc.vector.reciprocal(rsum[:], ssum_ps[:])

                # out[b,:] = (sum_c m[:,c] . f1[c*128: , :]) * rsum
                o_ps = psum_out_pool.tile([1, RNN], F32, tag="out")
                d_chunk = min(512, RNN)
                f1_half = LC // 2
                for half in range(2):
                    f1t = f1_pool.tile([P, f1_half, RNN], BF16, tag="f1")
                    nc.sync.dma_start(
                        f1t[:],
                        f1_d[b, half * f1_half * P : (half + 1) * f1_half * P].rearrange(
                            "(c p) d -> p c d", p=P
                        ),
                    )
                    for ci in range(f1_half):
                        c = half * f1_half + ci
                        w_col = mw_b[:, c : c + 1]
                        for dc in range(RNN // d_chunk):
                            nc.tensor.matmul(
                                o_ps[:, ts(dc, d_chunk)],
                                w_col,
                                f1t[:, ci, ts(dc, d_chunk)],
                                start=(c == 0),
                                stop=(c == LC - 1),
                            )
                # normalize during the PSUM->SBUF copy: out = in * (1/sum)
                nc.scalar.activation(
                    att_out[:, ts(b, RNN)],
                    o_ps[:],
                    AF.Copy,
                    scale=rsum[:],
                )

            nc.sync.dma_start(out_d.rearrange("b d -> (b d)")[None, :], att_out[:])

    nc.compile()
    return nc


_NC_CACHE = None


def _get_nc():
    global _NC_CACHE
    if _NC_CACHE is None:
        _NC_CACHE = build_nc()
    return _NC_CACHE


def _make_in_maps(inputs):
    import ml_dtypes

    bf = lambda x: np.ascontiguousarray(
        np.asarray(x, dtype=np.float32).astype(ml_dtypes.bfloat16)
    )
    f32 = lambda x: np.ascontiguousarray(np.asarray(x, dtype=np.float32))
    h = bf(inputs["h"])
    f1 = bf(inputs["att_feats1"])
    f2 = bf(inputs["att_feats2"])
    mask = f32(inputs["att_masks"])
    w = bf(inputs["W_h2att"])
    bh = bf(inputs["b_h2att"])
    wa = bf(inputs["w_alpha"])
    in_maps = []
    for i in range(N_CORES):
        sl = slice(i * BS, (i + 1) * BS)
        in_maps.append(
            {
                "h": h[sl],
                "att_feats1": f1[sl],
                "att_feats2": f2[sl],
                "att_masks": mask[sl],
                "W_h2att": w,
                "b_h2att": bh,
                "w_alpha": wa,
            }
        )
    return in_maps


def _ensure_ntff_hook():
    """The agent image's antenv lacks axon_hooks; shim it so trace=True can
    capture NTFF profiles through libaxon_pjrt's ctypes interface."""
    import sys
    import types

    try:
        import antenv.axon_hooks  # noqa: F401
        return
    except ImportError:
        pass
    try:
        from trn_agent_boot.trn_boot import _ntff_profile_via_ctypes

        hook = _ntff_profile_via_ctypes("/opt/axon/libaxon_pjrt.so")
    except Exception:
        hook = None
    mod = types.ModuleType("antenv.axon_hooks")
    mod._hook = hook
    mod.get_axon_ntff_profile_hook = lambda: mod._hook
    mod.set_axon_ntff_profile_hook = lambda h: setattr(mod, "_hook", h)
    sys.modules["antenv.axon_hooks"] = mod


def run(inputs, trace=False):
    """Returns (full_output [B, RNN] float32, exec_time_ns or None)."""
    if trace:
        _ensure_ntff_hook()
    nc = _get_nc()
    res = run_bass_kernel_spmd(
        nc, _make_in_maps(inputs), core_ids=list(range(N_CORES)), trace=trace
    )
    out = np.concatenate([r["out"] for r in res.results], axis=0)
    return out.astype(np.float32), res.exec_time_ns


def kernel(**inputs):
    out, _ = run(inputs, trace=False)
    return out


# revision 15
# speedup vs baseline: 2.4880x; 1.0591x over previous
"""Trainium2 Bass kernel for the masked-attention module.

Math (per batch row b):
    att_h = h @ W_h2att.T + b_h2att                       # [A]
    dot_l = sum_a tanh(f2[l,a] + att_h[a]) * w_alpha[a]   # [L]  (b_alpha cancels)
    m     = exp(dot) * mask      # softmax denominator cancels with masked renorm
    out   = (sum_l m[l] * f1[l,:]) / sum_l m[l]           # [D]

Sharding: data-parallel over B across 8 NeuronCores (16 rows each);
weights replicated.  Inputs are downcast to bf16 on the host (fp32 matmuls
run at 1/4 rate on TRN2 and fp32 doubles the DMA bytes); accumulations are
fp32.  Verified rel err vs the fp32 reference ~3e-3.
"""

import numpy as np

import concourse.bacc as bacc
import concourse.bass as bass
import concourse.mybir as mybir
import concourse.tile as tile
from concourse.bass import ts
from concourse.bass_utils import run_bass_kernel_spmd
from concourse.masks import make_identity

# Problem geometry (hardcoded per spec).
B, L, RNN, ATT = 128, 1024, 1024, 512
N_CORES = 8
BS = B // N_CORES          # 16 batch rows per core
P = 128                    # partitions
F32 = mybir.dt.float32
BF16 = mybir.dt.bfloat16
AF = mybir.ActivationFunctionType
ALU = mybir.AluOpType


def build_nc(BS=BS, L=L, RNN=RNN, ATT=ATT):
    import ml_dtypes

    LC = L // P            # l-chunks
    RC = RNN // P          # r-chunks
    AC = ATT // P          # a-chunks
    HLC = LC // 2          # l-chunks per half-batch piece
    nc = bacc.Bacc("TRN2", target_bir_lowering=False, debug=False)

    h_d = nc.dram_tensor("h", [BS, RNN], BF16, kind="ExternalInput").ap()
    f1_d = nc.dram_tensor("att_feats1", [BS, L, RNN], BF16, kind="ExternalInput").ap()
    f2_d = nc.dram_tensor("att_feats2", [BS, L, ATT], BF16, kind="ExternalInput").ap()
    mask_d = nc.dram_tensor("att_masks", [BS, L], F32, kind="ExternalInput").ap()
    w_d = nc.dram_tensor("W_h2att", [ATT, RNN], BF16, kind="ExternalInput").ap()
    bh_d = nc.dram_tensor("b_h2att", [ATT], BF16, kind="ExternalInput").ap()
    wa_d = nc.dram_tensor("w_alpha", [ATT], BF16, kind="ExternalInput").ap()
    out_d = nc.dram_tensor("out", [BS, RNN], F32, kind="ExternalOutput").ap()

    # delta[k, b*P + m] = (k == b): lhsT blocks that broadcast att_h row b
    # across all 128 output partitions in a single K=BS matmul.
    delta_np = np.zeros((BS, BS * P), dtype=ml_dtypes.bfloat16)
    for b in range(BS):
        delta_np[b, b * P : (b + 1) * P] = 1.0
    delta_d = nc.inline_tensor(delta_np, name="delta_bcast").ap()

    with tile.TileContext(nc) as tc:
        with (
            tc.tile_pool(name="singles", bufs=1) as singles,
            tc.tile_pool(name="wn", bufs=2) as wn_pool,
            tc.tile_pool(name="f2", bufs=5) as f2_pool,
            tc.tile_pool(name="f1", bufs=4) as f1_pool,
            tc.tile_pool(name="work", bufs=4) as work_pool,
            tc.tile_pool(name="small", bufs=3) as small_pool,
            tc.tile_pool(name="outp", bufs=3) as out_pool,
            tc.tile_pool(name="psum_misc", bufs=2, space="PSUM") as psum_misc,
            tc.tile_pool(name="psum_ahb", bufs=2, space="PSUM") as psum_ahb_pool,
            tc.tile_pool(name="psum_out", bufs=2, space="PSUM") as psum_out_pool,
        ):
            # ---------- constants ----------
            ident_bf = singles.tile([P, P], BF16)
            make_identity(nc, ident_bf[:])
            ident_f32 = singles.tile([P, P], F32)
            make_identity(nc, ident_f32[:])
            ones_row = singles.tile([1, P], BF16)   # K=1 bcast matmuls
            nc.vector.memset(ones_row[:], 1.0)
            ones_col = singles.tile([P, 1], F32)    # partition sums
            nc.vector.memset(ones_col[:], 1.0)
            delta_sb = singles.tile([BS, BS * P], BF16)
            nc.sync.dma_start(delta_sb[:], delta_d[:])

            wa_bcast = singles.tile([P, ATT], BF16)
            nc.sync.dma_start(wa_bcast[:], wa_d[None, :].to_broadcast((P, ATT)))
            bh_sb = singles.tile([1, ATT], BF16)
            nc.sync.dma_start(bh_sb[:], bh_d[None, :])

            # ---------- prologue: att_h = h @ W.T + bh  -> [BS, ATT] bf16 ----------
            # W^T in SBUF (bf16): WT[:, rc*ATT + a] = W[a, rc*128 + p]
            wt_all = singles.tile([P, RC * ATT], BF16)
            for ac in range(AC):
                wn = wn_pool.tile([P, RNN], BF16)
                nc.sync.dma_start(wn[:], w_d[ts(ac, P)])
                for rc in range(RC):
                    pt = psum_misc.tile([P, P], BF16, tag="misc")
                    nc.tensor.transpose(pt[:], wn[:, ts(rc, P)], ident_bf[:])
                    nc.scalar.copy(
                        wt_all[:, rc * ATT + ac * P : rc * ATT + (ac + 1) * P], pt[:]
                    )

            # h^T in SBUF (bf16): hT[:, rc*BS + b] = h[b, rc*128 + p]
            hn = singles.tile([BS, RNN], BF16)
            nc.sync.dma_start(hn[:], h_d[:])
            ht_all = singles.tile([P, RC * BS], BF16)
            for rc in range(RC):
                pt = psum_misc.tile([P, BS], BF16, tag="misc")
                nc.tensor.transpose(pt[:], hn[:, ts(rc, P)], ident_bf[:BS, :BS])
                nc.scalar.copy(ht_all[:, ts(rc, BS)], pt[:])

            # att_h[b, a] with M=BS, + bias via K=1 ones matmul
            ah_ps = psum_misc.tile([BS, ATT], F32, tag="misc")
            for rc in range(RC):
                nc.tensor.matmul(
                    ah_ps[:],
                    ht_all[:, ts(rc, BS)],
                    wt_all[:, ts(rc, ATT)],
                    start=(rc == 0),
                    stop=False,
                )
            nc.tensor.matmul(
                ah_ps[:], ones_row[:, :BS], bh_sb[:], start=False, stop=True
            )
            ah_sb = singles.tile([BS, ATT], BF16)
            nc.scalar.copy(ah_sb[:], ah_ps[:])

            # ---------- mask transpose: maskT[p, b*LC+c] = mask[b, c*128+p] ----------
            mask_nat = singles.tile([BS, L], F32)
            nc.sync.dma_start(mask_nat[:], mask_d[:])
            maskT = singles.tile([P, BS * LC], F32)
            maskT_3d = maskT[:].rearrange("p (b c) -> p b c", c=LC)
            for c in range(LC):
                pt = psum_misc.tile([P, BS], F32, tag="misc")
                nc.tensor.transpose(pt[:], mask_nat[:, ts(c, P)], ident_f32[:BS, :BS])
                nc.vector.tensor_copy(maskT_3d[:, :, c], pt[:])

            dot_all = singles.tile([P, BS * LC], F32)

            # ---------- per-batch pipeline ----------
            for b in range(BS):
                # broadcast att_h[b,:] to all partitions: delta-matmul, K=BS
                ahb_ps = psum_ahb_pool.tile([P, ATT], F32, tag="ahb")
                nc.tensor.matmul(
                    ahb_ps[:], delta_sb[:, ts(b, P)], ah_sb[:], start=True, stop=True
                )
                ahb = work_pool.tile([P, ATT], BF16, tag="ahb_sb")
                nc.scalar.copy(ahb[:], ahb_ps[:])

                # f2[b] in two 512 KiB DMAs: [128, HLC, ATT], l = c*128 + p
                for half in range(2):
                    f2t = f2_pool.tile([P, HLC, ATT], BF16, tag="f2")
                    nc.sync.dma_start(
                        f2t[:],
                        f2_d[b, half * HLC * P : (half + 1) * HLC * P].rearrange(
                            "(c p) a -> p c a", p=P
                        ),
                    )
                    t_add = work_pool.tile([P, HLC, ATT], BF16, tag="tadd")
                    nc.vector.tensor_add(
                        t_add[:],
                        f2t[:],
                        ahb[:][:, None, :].to_broadcast((P, HLC, ATT)),
                    )
                    t_tanh = work_pool.tile([P, HLC * ATT], BF16, tag="ttanh")
                    nc.scalar.activation(
                        t_tanh[:], t_add[:].rearrange("p c a -> p (c a)"), AF.Tanh
                    )
                    for ci in range(HLC):
                        c = half * HLC + ci
                        scratch = work_pool.tile([P, ATT], BF16, tag="scratch")
                        nc.vector.scalar_tensor_tensor(
                            out=scratch[:],
                            in0=t_tanh[:, ts(ci, ATT)],
                            scalar=1.0,
                            in1=wa_bcast[:],
                            op0=ALU.mult,
                            op1=ALU.mult,
                            accum_out=dot_all[:, b * LC + c : b * LC + c + 1],
                        )

                # masked exp (denominator cancels; normalization folded into
                # the output copy's scale)
                e_b = small_pool.tile([P, LC], F32, tag="eb")
                nc.scalar.activation(e_b[:], dot_all[:, ts(b, LC)], AF.Exp)
                m_b = small_pool.tile([P, LC], F32, tag="mb")
                nc.vector.tensor_mul(m_b[:], e_b[:], maskT[:, ts(b, LC)])
                mw_b = small_pool.tile([P, LC], BF16, tag="mwb")
                nc.vector.tensor_copy(mw_b[:], m_b[:])
                # sum over l: free reduce then partition sum via ones-matmul
                s_b = small_pool.tile([P, 1], F32, tag="sb")
                nc.vector.tensor_reduce(
                    s_b[:], m_b[:], axis=mybir.AxisListType.X, op=ALU.add
                )
                ssum_ps = psum_misc.tile([1, 1], F32, tag="misc")
                nc.tensor.matmul(ssum_ps[:], ones_col[:], s_b[:], start=True, stop=True)
                rsum = small_pool.tile([1, 1], F32, tag="rsum")
                nc.vector.reciprocal(rsum[:], ssum_ps[:])

                # out[b,:] = (sum_c m[:,c] . f1[c*128: , :]) * rsum
                o_ps = psum_out_pool.tile([1, RNN], F32, tag="out")
                d_chunk = min(512, RNN)
                for half in range(2):
                    f1t = f1_pool.tile([P, HLC, RNN], BF16, tag="f1")
                    nc.sync.dma_start(
                        f1t[:],
                        f1_d[b, half * HLC * P : (half + 1) * HLC * P].rearrange(
                            "(c p) d -> p c d", p=P
                        ),
                    )
                    for ci in range(HLC):
                        c = half * HLC + ci
                        w_col = mw_b[:, c : c + 1]
                        for dc in range(RNN // d_chunk):
                            nc.tensor.matmul(
                                o_ps[:, ts(dc, d_chunk)],
                                w_col,
                                f1t[:, ci, ts(dc, d_chunk)],
                                start=(c == 0),
                                stop=(c == LC - 1),
                            )
                # normalize during the PSUM->SBUF copy: out = in * (1/sum)
                o_sb = out_pool.tile([1, RNN], F32, tag="osb")
                nc.scalar.activation(o_sb[:], o_ps[:], AF.Copy, scale=rsum[:])
                nc.sync.dma_start(out_d[b][None, :], o_sb[:])

    nc.compile()
    return nc


_NC_CACHE = None


def _get_nc():
    global _NC_CACHE
    if _NC_CACHE is None:
        _NC_CACHE = build_nc()
    return _NC_CACHE


def _make_in_maps(inputs):
    import ml_dtypes

    bf = lambda x: np.ascontiguousarray(
        np.asarray(x, dtype=np.float32).astype(ml_dtypes.bfloat16)
    )
    f32 = lambda x: np.ascontiguousarray(np.asarray(x, dtype=np.float32))
    h = bf(inputs["h"])
    f1 = bf(inputs["att_feats1"])
    f2 = bf(inputs["att_feats2"])
    mask = f32(inputs["att_masks"])
    w = bf(inputs["W_h2att"])
    bh = bf(inputs["b_h2att"])
    wa = bf(inputs["w_alpha"])
    in_maps = []
    for i in range(N_CORES):
        sl = slice(i * BS, (i + 1) * BS)
        in_maps.append(
            {
                "h": h[sl],
                "att_feats1": f1[sl],
                "att_feats2": f2[sl],
                "att_masks": mask[sl],
                "W_h2att": w,
                "b_h2att": bh,
                "w_alpha": wa,
            }
        )
    return in_maps


def _ensure_ntff_hook():
    """The agent image's antenv lacks axon_hooks; shim it so trace=True can
    capture NTFF profiles through libaxon_pjrt's ctypes interface."""
    import sys
    import types

    try:
        import antenv.axon_hooks  # noqa: F401
        return
    except ImportError:
        pass
    try:
        from trn_agent_boot.trn_boot import _ntff_profile_via_ctypes

        hook = _ntff_profile_via_ctypes("/opt/axon/libaxon_pjrt.so")
    except Exception:
        hook = None
    mod = types.ModuleType("antenv.axon_hooks")
    mod._hook = hook
    mod.get_axon_ntff_profile_hook = lambda: mod._hook
    mod.set_axon_ntff_profile_hook = lambda h: setattr(mod, "_hook", h)
    sys.modules["antenv.axon_hooks"] = mod


def run(inputs, trace=False):
    """Returns (full_output [B, RNN] float32, exec_time_ns or None)."""
    if trace:
        _ensure_ntff_hook()
    nc = _get_nc()
    res = run_bass_kernel_spmd(
        nc, _make_in_maps(inputs), core_ids=list(range(N_CORES)), trace=trace
    )
    out = np.concatenate([r["out"] for r in res.results], axis=0)
    return out.astype(np.float32), res.exec_time_ns


def kernel(**inputs):
    out, _ = run(inputs, trace=False)
    return out
